# revision 2
# baseline (speedup 1.0000x reference)
"""nn_GateMulti — MoE routing (8 experts, one-hot gate) on 8 TRN2 NeuronCores.

Strategy: expert-parallel. The gate is exactly one-hot on groups[:, 0], so
each token needs exactly one expert's MLP. Host-side "all-to-all": sort the
4096 tokens by expert id, pad each expert's token set to a common capacity,
and hand core e exactly expert e's tokens (transposed) plus expert e's
weights. Each core then runs a dense 2-layer MLP:

    yT = W2.T @ relu(W1.T @ xT + b1) + b2        (feature-major layout)

All matmul operands are bf16 (half the DMA bytes, fast-weight-load); PSUM
accumulation is fp32. The PE stream is the roofline: 2*64 matmuls of
N=chunk columns. Everything else is arranged around keeping the PE fed:

- Input DMAs are issued in exact consumption order on the sync HWDGE ring
  (w1 f-tile-sliced so per-DMA completion tracks the layer-1 i-loop), with
  the xT pilot + biases on the scalar HWDGE ring so the two pilot pieces
  ramp concurrently. No gating: FIFO order within a ring already protects
  the critical bytes.
- A short burst of dependency-free warm-up matmuls covers the preamble->
  pilot window so the PE HAM clock gate reaches 8/8 around when real work
  starts, without delaying it.
- Layer-1 ReLU+bias tiles alternate between ScalarE (activation) and DVE
  (tensor_scalar add+max) so neither engine gates the PE's PSUM banks.
- y leaves in bf16 (host upcasts) packed [P, O_T, chunk] per chunk so
  out-DMA lines are 2 KB; the final o-tile goes out as its own small DMA
  on the scalar ring right after a split final ACT, shortening the tail.

The host scatters per-core outputs back to the original token order.
Self-contained: shapes hardcoded from the problem spec.
"""

import math
from functools import lru_cache

import ml_dtypes
import numpy as np

import concourse.bacc as bacc
import concourse.mybir as mybir
import concourse.tile as tile
from concourse.bass_utils import run_bass_kernel_spmd

E = 8
B = 4096
D_IN = 512
D_FF = 2048
D_OUT = 512
GROUP_COL = 0

P = 128
D_T = D_IN // P   # 4  k-tiles for layer 1
F_T = D_FF // P   # 16 f-tiles (layer-1 out / layer-2 contraction)
O_T = D_OUT // P  # 4  o-tiles for layer 2

F32 = mybir.dt.float32
BF16 = mybir.dt.bfloat16
W_DT = A_DT = BF16
W_NP = ml_dtypes.bfloat16

N_WARM = 11  # dependency-free scratch matmuls spanning preamble-end ->
             # pilot-DMA-landing (~2.5us at ~225ns cold each)

# w1 f-tile DMA batching: slice boundaries chosen so each DMA lands just
# ahead of the i-loop consuming it (early slices small, later ones bulk)
W1_SLICES = [(0, 1), (1, 2), (2, 3), (3, 6), (6, 10), (10, 16)]


def _emit(tc, nc, xT, w1, w2, bt, yT, cap, n_chunks, chunk):
    relu = mybir.ActivationFunctionType.Relu
    ident = mybir.ActivationFunctionType.Identity
    add = mybir.AluOpType.add
    amax = mybir.AluOpType.max

    with (
        tc.tile_pool(name="consts", bufs=1) as cpool,
        tc.tile_pool(name="acts", bufs=1) as apool,
        tc.tile_pool(name="yout", bufs=1) as ypool,
        tc.tile_pool(name="psum_h", bufs=4, space="PSUM") as ph,
        tc.tile_pool(name="psum_y", bufs=4, space="PSUM") as py,
    ):
        # ---- PE warm-up: scratch matmuls with no input dependencies. They
        # hold the HAM activity window busy during the pilot-DMA wait so the
        # real stream reaches 2.4 GHz quickly, sized to not overshoot it.
        warm_w = cpool.tile([P, P], W_DT)
        warm_x = cpool.tile([P, chunk], A_DT)
        nc.gpsimd.memset(warm_w[:], 0.0)
        nc.gpsimd.memset(warm_x[:], 0.0)
        warm_p = py.tile([P, chunk], F32, name="warm_p", tag="yp")
        for _ in range(N_WARM):
            nc.tensor.matmul(warm_p[:], warm_w[:], warm_x[:])

        # ---- input DMAs, issued in consumption order. Sync ring carries
        # the w1 stream (f-tile batches sized so each completion lands just
        # ahead of its i-loop consumer) then w2; scalar ring carries the xT
        # pilot + biases concurrently. No cross-ring gating needed: within
        # a ring the FIFO paces everything behind the pilot bytes.
        w1_sb = cpool.tile([P, F_T, D_T, P], W_DT)   # [p, i, j, c]
        w2_sb = cpool.tile([P, O_T, F_T, P], W_DT)   # [p, k, i, c]
        xT_sb = apool.tile([P, n_chunks, D_T, chunk], A_DT)
        bt_sb = cpool.tile([P, F_T + O_T], F32)

        nc.scalar.dma_start(xT_sb[:, 0], xT.ap()[0])
        nc.scalar.dma_start(bt_sb[:], bt.ap())
        for c in range(1, n_chunks):
            nc.scalar.dma_start(xT_sb[:, c], xT.ap()[c])
        for lo, hi in W1_SLICES:
            nc.sync.dma_start(w1_sb[:, lo:hi], w1.ap()[:, lo:hi])
        nc.sync.dma_start(w2_sb[:, 0:2], w2.ap()[:, 0:2])
        nc.sync.dma_start(w2_sb[:, 2:4], w2.ap()[:, 2:4])

        hT_sb = apool.tile([P, F_T, cap], A_DT)

        # ---- layer 1: hT[f, c] = relu(sum_d W1[d, f] xT[d, c] + b1[f])
        # chunk-interleaved so each w1 i-slice is consumed over 2x the time;
        # ReLU tiles alternate ScalarE / DVE so neither engine backs up PSUM
        for i in range(F_T):
            for c in range(n_chunks):
                cs = slice(c * chunk, (c + 1) * chunk)
                hp = ph.tile([P, chunk], F32, name=f"hp_{i}_{c}", tag="hp")
                for j in range(D_T):
                    nc.tensor.matmul(
                        hp[:],
                        w1_sb[:, i, j, :],
                        xT_sb[:, c, j, :],
                        start=(j == 0),
                        stop=(j == D_T - 1),
                    )
                if (i + c) % 2 == 0:
                    nc.scalar.activation(
                        hT_sb[:, i, cs], hp[:], relu, bias=bt_sb[:, i : i + 1]
                    )
                else:
                    nc.vector.tensor_scalar(
                        hT_sb[:, i, cs], hp[:], bt_sb[:, i : i + 1], 0.0, add, amax
                    )

        # ---- layer 2: yT[o, c] = sum_f W2[f, o] hT[f, c] + b2[o]
        # c outer so each chunk's 4 o-tiles pack into one [P, O_T, chunk]
        # SBUF tile -> one 2KB-line DMA per chunk. The final o-tile of the
        # last chunk is ACT-split across ScalarE+DVE and leaves as its own
        # small DMA on the scalar ring so the tail is as short as possible.
        for c in range(n_chunks):
            cs = slice(c * chunk, (c + 1) * chunk)
            yo = ypool.tile([P, O_T, chunk], A_DT, name=f"yo_{c}", tag=f"yo{c}")
            last_c = c == n_chunks - 1
            for k in range(O_T):
                yp = py.tile([P, chunk], F32, name=f"yp_{k}_{c}", tag="yp")
                for i in range(F_T):
                    nc.tensor.matmul(
                        yp[:],
                        w2_sb[:, k, i, :],
                        hT_sb[:, i, cs],
                        start=(i == 0),
                        stop=(i == F_T - 1),
                    )
                b2c = bt_sb[:, F_T + k : F_T + k + 1]
                last = last_c and k == O_T - 1
                if not last:
                    nc.vector.tensor_scalar_add(yo[:, k], yp[:], b2c)
                else:
                    # final tile: two half-width bias-adds on both engines
                    # concurrently so its DMA can issue as early as possible
                    half = chunk // 2
                    nc.scalar.activation(
                        yo[:, k, 0:half], yp[:, 0:half], ident, bias=b2c
                    )
                    nc.vector.tensor_scalar_add(
                        yo[:, k, half:chunk], yp[:, half:chunk], b2c
                    )
            if not last_c:
                nc.sync.dma_start(yT[c], yo[:])
            else:
                # split the last chunk's out-DMA: o-tiles 0..2 on sync once
                # ready, the final o-tile alone on the scalar ring
                nc.sync.dma_start(yT[c, :, 0 : O_T - 1], yo[:, 0 : O_T - 1])
                nc.scalar.dma_start(yT[c, :, O_T - 1 :], yo[:, O_T - 1 :])


@lru_cache(maxsize=4)
def _build_nc(cap, n_chunks, chunk):
    nc = bacc.Bacc("TRN2", target_bir_lowering=False, debug=False, num_devices=E)
    xT = nc.dram_tensor("xT", [n_chunks, P, D_T, chunk], A_DT, kind="ExternalInput")
    w1 = nc.dram_tensor("w1", [P, F_T, D_T, P], W_DT, kind="ExternalInput")
    w2 = nc.dram_tensor("w2", [P, O_T, F_T, P], W_DT, kind="ExternalInput")
    bt = nc.dram_tensor("bt", [P, F_T + O_T], F32, kind="ExternalInput")
    yT = nc.dram_tensor("yT", [n_chunks, P, O_T, chunk], A_DT, kind="ExternalOutput")
    with tile.TileContext(nc) as tc:
        _emit(tc, nc, xT, w1, w2, bt, yT, cap, n_chunks, chunk)
    nc.compile()
    return nc


def _plan_capacity(max_count):
    cap0 = max(int(max_count), 16)
    n_chunks = max(1, math.ceil(cap0 / 512))
    chunk = math.ceil(cap0 / (n_chunks * 2)) * 2
    return n_chunks * chunk, n_chunks, chunk


def _pack_w1(W1e):
    # w1img[p, i, j, c] = W1e[j*128 + p, i*128 + c]
    return np.ascontiguousarray(
        W1e.reshape(D_T, P, F_T, P).transpose(1, 2, 0, 3).astype(W_NP)
    )


def _pack_w2(W2e):
    # w2img[p, k, i, c] = W2e[i*128 + p, k*128 + c]
    return np.ascontiguousarray(
        W2e.reshape(F_T, P, O_T, P).transpose(1, 2, 0, 3).astype(W_NP)
    )


def _shard(x, groups, W1, b1, W2, b2):
    idx = np.asarray(groups)[:, GROUP_COL].astype(np.int64)
    order = np.argsort(idx, kind="stable")
    counts = np.bincount(idx, minlength=E)
    cap, n_chunks, chunk = _plan_capacity(counts.max())
    offs = np.concatenate([[0], np.cumsum(counts)])

    x = np.asarray(x, dtype=np.float32)
    W1 = np.asarray(W1, dtype=np.float32)
    b1 = np.asarray(b1, dtype=np.float32)
    W2 = np.asarray(W2, dtype=np.float32)
    b2 = np.asarray(b2, dtype=np.float32)

    in_maps, tok_ids = [], []
    for e in range(E):
        ids = order[offs[e] : offs[e + 1]]
        tok_ids.append(ids)
        xT = np.zeros((D_IN, cap), np.float32)
        xT[:, : len(ids)] = x[ids].T
        # pack each chunk to its SBUF image [p, j, c] so every chunk DMA
        # moves multi-KB contiguous lines on both sides
        xT = np.ascontiguousarray(
            xT.reshape(D_T, P, n_chunks, chunk)
            .transpose(2, 1, 0, 3)
            .astype(W_NP)
        )
        bt = np.concatenate(
            [b1[e].reshape(F_T, P).T, b2[e].reshape(O_T, P).T], axis=1
        )
        in_maps.append(
            {
                "xT": xT,
                "w1": _pack_w1(W1[e]),
                "w2": _pack_w2(W2[e]),
                "bt": np.ascontiguousarray(bt),
            }
        )
    return in_maps, tok_ids, counts, cap, n_chunks, chunk


def _run(x, groups, W1, b1, W2, b2, trace=False, **spmd_kwargs):
    in_maps, tok_ids, counts, cap, n_chunks, chunk = _shard(x, groups, W1, b1, W2, b2)
    nc = _build_nc(cap, n_chunks, chunk)
    res = run_bass_kernel_spmd(
        nc, in_maps, core_ids=list(range(E)), trace=trace, **spmd_kwargs
    )
    out = np.zeros((B, D_OUT), np.float32)
    for e in range(E):
        # yT[c, p, k, cc] = y[k*128 + p, c*chunk + cc]
        yTe = np.asarray(res.results[e]["yT"], dtype=np.float32)
        ye = yTe.transpose(2, 1, 0, 3).reshape(D_OUT, cap)
        out[tok_ids[e]] = ye[:, : counts[e]].T
    return out, res


def kernel(x, groups, W1, b1, W2, b2):
    out, _ = _run(x, groups, W1, b1, W2, b2)
    return out


# revision 3
# speedup vs baseline: 1.0863x; 1.0863x over previous
"""nn_GateMulti — MoE routing (8 experts, one-hot gate) on 8 TRN2 NeuronCores.

Strategy: expert-parallel. The gate is exactly one-hot on groups[:, 0], so
each token needs exactly one expert's MLP. Host-side "all-to-all": sort the
4096 tokens by expert id, pad each expert's token set to a common capacity,
and hand core e exactly expert e's tokens (transposed) plus expert e's
weights. Each core then runs a dense 2-layer MLP:

    yT = W2.T @ relu(W1.T @ xT + b1) + b2        (feature-major layout)

All matmul operands are bf16 (half the DMA bytes, fast-weight-load); PSUM
accumulation is fp32. The PE stream is the roofline: 2*64 matmuls of
N=chunk columns. Everything else is arranged around keeping the PE fed:

- Input DMAs are issued in exact consumption order on the sync HWDGE ring
  (w1 f-tile-sliced so per-DMA completion tracks the layer-1 i-loop), with
  the xT pilot + biases on the scalar HWDGE ring so the two pilot pieces
  ramp concurrently. No gating: FIFO order within a ring already protects
  the critical bytes.
- A short burst of dependency-free warm-up matmuls covers the preamble->
  pilot window so the PE HAM clock gate reaches 8/8 around when real work
  starts, without delaying it.
- Layer-1 ReLU+bias tiles alternate between ScalarE (activation) and DVE
  (tensor_scalar add+max) so neither engine gates the PE's PSUM banks.
- y leaves in bf16 (host upcasts) packed [P, O_T, chunk] per chunk so
  out-DMA lines are 2 KB; the final o-tile goes out as its own small DMA
  on the scalar ring right after a split final ACT, shortening the tail.

The host scatters per-core outputs back to the original token order.
Self-contained: shapes hardcoded from the problem spec.
"""

import math
from functools import lru_cache

import ml_dtypes
import numpy as np

import concourse.bacc as bacc
import concourse.mybir as mybir
import concourse.tile as tile
from concourse.bass_utils import run_bass_kernel_spmd

E = 8
B = 4096
D_IN = 512
D_FF = 2048
D_OUT = 512
GROUP_COL = 0

P = 128
D_T = D_IN // P   # 4  k-tiles for layer 1
F_T = D_FF // P   # 16 f-tiles (layer-1 out / layer-2 contraction)
O_T = D_OUT // P  # 4  o-tiles for layer 2

F32 = mybir.dt.float32
BF16 = mybir.dt.bfloat16
W_DT = A_DT = BF16
W_NP = ml_dtypes.bfloat16

N_WARM = 11  # dependency-free scratch matmuls spanning preamble-end ->
             # pilot-DMA-landing (~2.5us at ~225ns cold each)

# w1 f-tile DMA batching: slice boundaries chosen so each DMA lands just
# ahead of the i-loop consuming it (early slices small, later ones bulk)
W1_SLICES = [(0, 1), (1, 2), (2, 3), (3, 6), (6, 10), (10, 16)]


def _emit(tc, nc, xT, w1, w2, bt, yT, cap, n_chunks, chunk):
    relu = mybir.ActivationFunctionType.Relu
    ident = mybir.ActivationFunctionType.Identity
    add = mybir.AluOpType.add
    amax = mybir.AluOpType.max

    with (
        tc.tile_pool(name="consts", bufs=1) as cpool,
        tc.tile_pool(name="acts", bufs=1) as apool,
        tc.tile_pool(name="yout", bufs=1) as ypool,
        tc.tile_pool(name="psum_h", bufs=4, space="PSUM") as ph,
        tc.tile_pool(name="psum_y", bufs=4, space="PSUM") as py,
    ):
        # ---- PE warm-up: scratch matmuls with no input dependencies. They
        # hold the HAM activity window busy during the pilot-DMA wait so the
        # real stream reaches 2.4 GHz quickly, sized to not overshoot it.
        warm_w = cpool.tile([P, P], W_DT)
        warm_x = cpool.tile([P, chunk], A_DT)
        nc.gpsimd.memset(warm_w[:], 0.0)
        nc.gpsimd.memset(warm_x[:], 0.0)
        warm_p = py.tile([P, chunk], F32, name="warm_p", tag="yp")
        for _ in range(N_WARM):
            nc.tensor.matmul(warm_p[:], warm_w[:], warm_x[:])

        # ---- input DMAs, issued in consumption order. Sync ring carries
        # the w1 stream (f-tile batches sized so each completion lands just
        # ahead of its i-loop consumer) then w2; scalar ring carries the xT
        # pilot + biases concurrently. No cross-ring gating needed: within
        # a ring the FIFO paces everything behind the pilot bytes.
        w1_sb = cpool.tile([P, F_T, D_T, P], W_DT)   # [p, i, j, c]
        w2_sb = cpool.tile([P, O_T, F_T, P], W_DT)   # [p, k, i, c]
        xT_sb = apool.tile([P, n_chunks, D_T, chunk], A_DT)
        bt_sb = cpool.tile([P, F_T + O_T], F32)

        nc.scalar.dma_start(bt_sb[:], bt.ap())
        nc.sync.dma_start(w1_sb[:, 0:1], w1.ap()[:, 0:1])
        for c in range(n_chunks):
            nc.sync.dma_start(xT_sb[:, c], xT.ap()[c])
        for lo, hi in W1_SLICES[1:]:
            nc.sync.dma_start(w1_sb[:, lo:hi], w1.ap()[:, lo:hi])
        nc.sync.dma_start(w2_sb[:, 0:2], w2.ap()[:, 0:2])
        nc.sync.dma_start(w2_sb[:, 2:4], w2.ap()[:, 2:4])

        hT_sb = apool.tile([P, F_T, cap], A_DT)

        # ---- layer 1: hT[f, c] = relu(sum_d W1[d, f] xT[d, c] + b1[f])
        # chunk-interleaved so each w1 i-slice is consumed over 2x the time;
        # ReLU tiles alternate ScalarE / DVE so neither engine backs up PSUM
        for i in range(F_T):
            for c in range(n_chunks):
                cs = slice(c * chunk, (c + 1) * chunk)
                hp = ph.tile([P, chunk], F32, name=f"hp_{i}_{c}", tag="hp")
                for j in range(D_T):
                    nc.tensor.matmul(
                        hp[:],
                        w1_sb[:, i, j, :],
                        xT_sb[:, c, j, :],
                        start=(j == 0),
                        stop=(j == D_T - 1),
                    )
                if (i + c) % 2 == 0:
                    nc.scalar.activation(
                        hT_sb[:, i, cs], hp[:], relu, bias=bt_sb[:, i : i + 1]
                    )
                else:
                    nc.vector.tensor_scalar(
                        hT_sb[:, i, cs], hp[:], bt_sb[:, i : i + 1], 0.0, add, amax
                    )

        # ---- layer 2: yT[o, c] = sum_f W2[f, o] hT[f, c] + b2[o]
        # c outer so each chunk's 4 o-tiles pack into one [P, O_T, chunk]
        # SBUF tile -> one 2KB-line DMA per chunk. The final o-tile of the
        # last chunk is ACT-split across ScalarE+DVE and leaves as its own
        # small DMA on the scalar ring so the tail is as short as possible.
        for c in range(n_chunks):
            cs = slice(c * chunk, (c + 1) * chunk)
            yo = ypool.tile([P, O_T, chunk], A_DT, name=f"yo_{c}", tag=f"yo{c}")
            last_c = c == n_chunks - 1
            for k in range(O_T):
                yp = py.tile([P, chunk], F32, name=f"yp_{k}_{c}", tag="yp")
                for i in range(F_T):
                    nc.tensor.matmul(
                        yp[:],
                        w2_sb[:, k, i, :],
                        hT_sb[:, i, cs],
                        start=(i == 0),
                        stop=(i == F_T - 1),
                    )
                b2c = bt_sb[:, F_T + k : F_T + k + 1]
                last = last_c and k == O_T - 1
                if not last:
                    nc.vector.tensor_scalar_add(yo[:, k], yp[:], b2c)
                else:
                    # final tile: two half-width bias-adds on both engines
                    # concurrently so its DMA can issue as early as possible
                    half = chunk // 2
                    nc.scalar.activation(
                        yo[:, k, 0:half], yp[:, 0:half], ident, bias=b2c
                    )
                    nc.vector.tensor_scalar_add(
                        yo[:, k, half:chunk], yp[:, half:chunk], b2c
                    )
            if not last_c:
                nc.sync.dma_start(yT[c], yo[:])
            else:
                # split the last chunk's out-DMA: o-tiles 0..2 on sync once
                # ready, the final o-tile alone on the scalar ring
                nc.sync.dma_start(yT[c, :, 0 : O_T - 1], yo[:, 0 : O_T - 1])
                nc.scalar.dma_start(yT[c, :, O_T - 1 :], yo[:, O_T - 1 :])


@lru_cache(maxsize=4)
def _build_nc(cap, n_chunks, chunk):
    nc = bacc.Bacc("TRN2", target_bir_lowering=False, debug=False, num_devices=E)
    xT = nc.dram_tensor("xT", [n_chunks, P, D_T, chunk], A_DT, kind="ExternalInput")
    w1 = nc.dram_tensor("w1", [P, F_T, D_T, P], W_DT, kind="ExternalInput")
    w2 = nc.dram_tensor("w2", [P, O_T, F_T, P], W_DT, kind="ExternalInput")
    bt = nc.dram_tensor("bt", [P, F_T + O_T], F32, kind="ExternalInput")
    yT = nc.dram_tensor("yT", [n_chunks, P, O_T, chunk], A_DT, kind="ExternalOutput")
    with tile.TileContext(nc) as tc:
        _emit(tc, nc, xT, w1, w2, bt, yT, cap, n_chunks, chunk)
    nc.compile()
    return nc


def _plan_capacity(max_count):
    cap0 = max(int(max_count), 16)
    n_chunks = max(1, math.ceil(cap0 / 512))
    chunk = math.ceil(cap0 / (n_chunks * 2)) * 2
    return n_chunks * chunk, n_chunks, chunk


def _pack_w1(W1e):
    # w1img[p, i, j, c] = W1e[j*128 + p, i*128 + c]
    return np.ascontiguousarray(
        W1e.reshape(D_T, P, F_T, P).transpose(1, 2, 0, 3).astype(W_NP)
    )


def _pack_w2(W2e):
    # w2img[p, k, i, c] = W2e[i*128 + p, k*128 + c]
    return np.ascontiguousarray(
        W2e.reshape(F_T, P, O_T, P).transpose(1, 2, 0, 3).astype(W_NP)
    )


def _shard(x, groups, W1, b1, W2, b2):
    idx = np.asarray(groups)[:, GROUP_COL].astype(np.int64)
    order = np.argsort(idx, kind="stable")
    counts = np.bincount(idx, minlength=E)
    cap, n_chunks, chunk = _plan_capacity(counts.max())
    offs = np.concatenate([[0], np.cumsum(counts)])

    x = np.asarray(x, dtype=np.float32)
    W1 = np.asarray(W1, dtype=np.float32)
    b1 = np.asarray(b1, dtype=np.float32)
    W2 = np.asarray(W2, dtype=np.float32)
    b2 = np.asarray(b2, dtype=np.float32)

    in_maps, tok_ids = [], []
    for e in range(E):
        ids = order[offs[e] : offs[e + 1]]
        tok_ids.append(ids)
        xT = np.zeros((D_IN, cap), np.float32)
        xT[:, : len(ids)] = x[ids].T
        # pack each chunk to its SBUF image [p, j, c] so every chunk DMA
        # moves multi-KB contiguous lines on both sides
        xT = np.ascontiguousarray(
            xT.reshape(D_T, P, n_chunks, chunk)
            .transpose(2, 1, 0, 3)
            .astype(W_NP)
        )
        bt = np.concatenate(
            [b1[e].reshape(F_T, P).T, b2[e].reshape(O_T, P).T], axis=1
        )
        in_maps.append(
            {
                "xT": xT,
                "w1": _pack_w1(W1[e]),
                "w2": _pack_w2(W2[e]),
                "bt": np.ascontiguousarray(bt),
            }
        )
    return in_maps, tok_ids, counts, cap, n_chunks, chunk


def _run(x, groups, W1, b1, W2, b2, trace=False, **spmd_kwargs):
    in_maps, tok_ids, counts, cap, n_chunks, chunk = _shard(x, groups, W1, b1, W2, b2)
    nc = _build_nc(cap, n_chunks, chunk)
    res = run_bass_kernel_spmd(
        nc, in_maps, core_ids=list(range(E)), trace=trace, **spmd_kwargs
    )
    out = np.zeros((B, D_OUT), np.float32)
    for e in range(E):
        # yT[c, p, k, cc] = y[k*128 + p, c*chunk + cc]
        yTe = np.asarray(res.results[e]["yT"], dtype=np.float32)
        ye = yTe.transpose(2, 1, 0, 3).reshape(D_OUT, cap)
        out[tok_ids[e]] = ye[:, : counts[e]].T
    return out, res


def kernel(x, groups, W1, b1, W2, b2):
    out, _ = _run(x, groups, W1, b1, W2, b2)
    return out


# revision 27
# speedup vs baseline: 1.1210x; 1.0320x over previous
"""nn_GateMulti — MoE routing (8 experts, one-hot gate) on 8 TRN2 NeuronCores.

Strategy: expert-parallel. The gate is exactly one-hot on groups[:, 0], so
each token needs exactly one expert's MLP. Host-side "all-to-all": sort the
4096 tokens by expert id, pad each expert's token set to a common capacity,
and hand core e exactly expert e's tokens (transposed) plus expert e's
weights. Each core then runs a dense 2-layer MLP:

    yT = W2.T @ relu(W1.T @ xT + b1) + b2        (feature-major layout)

All matmul operands are bf16 (half the DMA bytes, fast-weight-load); PSUM
accumulation is fp32. The PE stream is the roofline: 2*128 matmuls of
N=chunk columns at ~N/2.4GHz+3ns each. Everything else is arranged around
keeping the PE fed:

- Input DMAs go on the sync HWDGE ring in exact consumption order and are
  chained (transfer k waits for transfer k-3): queued DMAs round-robin at
  packet granularity across the 16 SDMA engines, so an unchained deep
  queue stretches every transfer's completion semaphore by multiple us.
  Chaining on the DMAs themselves keeps delivery at wire speed regardless
  of PE pace. Biases ride the scalar ring.
- A burst of dependency-free warm-up matmuls covers the preamble->pilot
  window so the PE HAM clock gate reaches 8/8 when real work starts, and
  cushions wire-ramp variance (a late pilot otherwise idles the PE long
  enough for the HAM MID window to re-throttle the early stream).
- Layer 1 runs chunk-interleaved with the first two f-tiles chunk-0-first,
  so xT chunk 1 (3rd transfer on the wire) is not needed until ~2
  tile-times into the stream. ReLU+bias tiles alternate between ScalarE
  (activation) and DVE (tensor_scalar add+max) so neither engine gates
  the PE's PSUM banks.
- y leaves in bf16 (host upcasts) packed [P, O_T, chunk] per chunk so
  out-DMA lines are 2 KB; the final o-tile goes out as its own small DMA
  on the scalar ring right after a ScalarE-only final ACT, keeping the
  tail to ACT + one issue + receipt.

The host scatters per-core outputs back to the original token order.
Self-contained: shapes hardcoded from the problem spec.
"""

import math
from functools import lru_cache

import ml_dtypes
import numpy as np

import concourse.bacc as bacc
import concourse.mybir as mybir
import concourse.tile as tile
from concourse.bass_utils import run_bass_kernel_spmd

E = 8
B = 4096
D_IN = 512
D_FF = 2048
D_OUT = 512
GROUP_COL = 0

P = 128
D_T = D_IN // P   # 4  k-tiles for layer 1
F_T = D_FF // P   # 16 f-tiles (layer-1 out / layer-2 contraction)
O_T = D_OUT // P  # 4  o-tiles for layer 2

F32 = mybir.dt.float32
BF16 = mybir.dt.bfloat16
W_DT = A_DT = BF16
W_NP = ml_dtypes.bfloat16

N_WARM = 16  # dependency-free scratch matmuls spanning preamble-end ->
             # pilot-sem-firing (~10.5-11us at ~226ns cold each); also
             # lifts the HAM clock gate to 8/8 by the time the real stream
             # starts, and cushions run-to-run wire-ramp variance (a late
             # pilot otherwise leaves the PE idle long enough for the HAM
             # MID window to re-throttle the early stream to 1.2 GHz)

# w1 f-tile DMA slicing. Input DMAs are chained: transfer k waits for
# transfer k-CHAIN_DEPTH to complete, so at most ~3 share the wire at
# once. Queued DMAs round-robin at packet granularity across the 16 SDMA
# engines, so an unchained deep queue stretches every transfer's
# completion by multiple us; chaining on the DMAs themselves (rather than
# compute progress) keeps delivery at wire speed regardless of PE pace.
W1_SLICES = [(0, 2), (2, 4), (4, 6), (6, 8), (8, 11), (11, 16)]
CHAIN_DEPTH = 3


def _emit(tc, nc, xT, w1, w2, bt, yT, cap, n_chunks, chunk):
    relu = mybir.ActivationFunctionType.Relu
    ident = mybir.ActivationFunctionType.Identity
    add = mybir.AluOpType.add
    amax = mybir.AluOpType.max
    from concourse.bass import _add_dep_helper

    with (
        tc.tile_pool(name="consts", bufs=1) as cpool,
        tc.tile_pool(name="acts", bufs=1) as apool,
        tc.tile_pool(name="yout", bufs=1) as ypool,
        tc.tile_pool(name="psum_h", bufs=4, space="PSUM") as ph,
        tc.tile_pool(name="psum_y", bufs=4, space="PSUM") as py,
    ):
        # ---- PE warm-up: scratch matmuls with no input dependencies. They
        # hold the HAM activity window busy during the pilot-DMA wait so the
        # real stream reaches 2.4 GHz quickly, sized to not overshoot it.
        warm_w = cpool.tile([P, P], W_DT)
        warm_x = cpool.tile([P, chunk], A_DT)
        nc.vector.memset(warm_w[:], 0.0)
        nc.vector.memset(warm_x[:], 0.0)
        warm_p = py.tile([P, chunk], F32, name="warm_p", tag="yp")
        for _ in range(N_WARM):
            nc.tensor.matmul(warm_p[:], warm_w[:], warm_x[:])

        # ---- input DMAs, issued in consumption order. Sync ring carries
        # the w1 stream (f-tile batches sized so each completion lands just
        # ahead of its i-loop consumer) then w2; scalar ring carries the xT
        # pilot + biases concurrently. No cross-ring gating needed: within
        # a ring the FIFO paces everything behind the pilot bytes.
        w1_sb = cpool.tile([P, F_T, D_T, P], W_DT)   # [p, i, j, c]
        w2_sb = cpool.tile([P, O_T, F_T, P], W_DT)   # [p, k, i, c]
        xT_sb = apool.tile([P, n_chunks, D_T, chunk], A_DT)
        bt_sb = cpool.tile([P, F_T + O_T], F32)

        nc.scalar.dma_start(bt_sb[:], bt.ap())
        chain = []
        chain.append(nc.sync.dma_start(xT_sb[:, 0], xT.ap()[0]))
        chain.append(nc.sync.dma_start(w1_sb[:, 0:2], w1.ap()[:, 0:2]))
        for c in range(1, n_chunks):
            chain.append(nc.sync.dma_start(xT_sb[:, c], xT.ap()[c]))
        for lo, hi in W1_SLICES[1:]:
            chain.append(nc.sync.dma_start(w1_sb[:, lo:hi], w1.ap()[:, lo:hi]))
        chain.append(nc.sync.dma_start(w2_sb[:, 0:2], w2.ap()[:, 0:2]))
        chain.append(nc.sync.dma_start(w2_sb[:, 2:4], w2.ap()[:, 2:4]))
        for k in range(CHAIN_DEPTH, len(chain)):
            _add_dep_helper(
                chain[k].ins,
                chain[k - CHAIN_DEPTH].ins,
                sync=True,
                reason="cap in-flight input DMAs at CHAIN_DEPTH",
            )

        hT_sb = apool.tile([P, F_T, cap], A_DT)

        # ---- layer 1: hT[f, c] = relu(sum_d W1[d, f] xT[d, c] + b1[f])
        # chunk-interleaved (i outer) so each w1 f-tile is consumed at half
        # rate, keeping every w1 DMA's completion comfortably ahead of its
        # consumer; the first two i-tiles run chunk-0-first so xT chunk 1
        # is not needed until ~2 tile-times into the stream (it is the 3rd
        # transfer on the wire). ReLU tiles alternate ScalarE / DVE so
        # neither engine backs up PSUM.
        l1_order = [(i, c) for i in range(F_T) for c in range(n_chunks)]
        if F_T >= 2 and n_chunks == 2:
            l1_order[0:4] = [(0, 0), (1, 0), (0, 1), (1, 1)]
        for i, c in l1_order:
            cs = slice(c * chunk, (c + 1) * chunk)
            hp = ph.tile([P, chunk], F32, name=f"hp_{i}_{c}", tag="hp")
            for j in range(D_T):
                nc.tensor.matmul(
                    hp[:],
                    w1_sb[:, i, j, :],
                    xT_sb[:, c, j, :],
                    start=(j == 0),
                    stop=(j == D_T - 1),
                )
            if (i + c) % 2 == 0:
                nc.scalar.activation(
                    hT_sb[:, i, cs], hp[:], relu, bias=bt_sb[:, i : i + 1]
                )
            else:
                nc.vector.tensor_scalar(
                    hT_sb[:, i, cs], hp[:], bt_sb[:, i : i + 1], 0.0, add, amax
                )

        # ---- layer 2: yT[o, c] = sum_f W2[f, o] hT[f, c] + b2[o]
        # c outer so each chunk's 4 o-tiles pack into one [P, O_T, chunk]
        # SBUF tile -> one 2KB-line DMA per chunk. The final o-tile of the
        # last chunk is ACT-split across ScalarE+DVE and leaves as its own
        # small DMA on the scalar ring so the tail is as short as possible.
        for c in range(n_chunks):
            cs = slice(c * chunk, (c + 1) * chunk)
            yo = ypool.tile([P, O_T, chunk], A_DT, name=f"yo_{c}", tag=f"yo{c}")
            last_c = c == n_chunks - 1
            for k in range(O_T):
                yp = py.tile([P, chunk], F32, name=f"yp_{k}_{c}", tag="yp")
                for i in range(F_T):
                    nc.tensor.matmul(
                        yp[:],
                        w2_sb[:, k, i, :],
                        hT_sb[:, i, cs],
                        start=(i == 0),
                        stop=(i == F_T - 1),
                    )
                b2c = bt_sb[:, F_T + k : F_T + k + 1]
                last = last_c and k == O_T - 1
                if not last:
                    nc.vector.tensor_scalar_add(yo[:, k], yp[:], b2c)
                else:
                    # final tile on ScalarE alone: its DMA follows on the
                    # same queue with no cross-engine semaphore hop
                    nc.scalar.activation(yo[:, k], yp[:], ident, bias=b2c)
            if not last_c:
                nc.sync.dma_start(yT[c], yo[:])
            else:
                # split the last chunk's out-DMA: o-tiles 0..2 on sync once
                # ready, the final o-tile alone on the scalar ring
                nc.sync.dma_start(yT[c, :, 0 : O_T - 1], yo[:, 0 : O_T - 1])
                nc.scalar.dma_start(yT[c, :, O_T - 1 :], yo[:, O_T - 1 :])


@lru_cache(maxsize=4)
def _build_nc(cap, n_chunks, chunk):
    nc = bacc.Bacc("TRN2", target_bir_lowering=False, debug=False, num_devices=E)
    xT = nc.dram_tensor("xT", [n_chunks, P, D_T, chunk], A_DT, kind="ExternalInput")
    w1 = nc.dram_tensor("w1", [P, F_T, D_T, P], W_DT, kind="ExternalInput")
    w2 = nc.dram_tensor("w2", [P, O_T, F_T, P], W_DT, kind="ExternalInput")
    bt = nc.dram_tensor("bt", [P, F_T + O_T], F32, kind="ExternalInput")
    yT = nc.dram_tensor("yT", [n_chunks, P, O_T, chunk], A_DT, kind="ExternalOutput")
    with tile.TileContext(nc) as tc:
        _emit(tc, nc, xT, w1, w2, bt, yT, cap, n_chunks, chunk)
    nc.compile()
    return nc


def _plan_capacity(max_count):
    cap0 = max(int(max_count), 16)
    n_chunks = max(1, math.ceil(cap0 / 512))
    chunk = math.ceil(cap0 / (n_chunks * 2)) * 2
    return n_chunks * chunk, n_chunks, chunk


def _pack_w1(W1e):
    # w1img[p, i, j, c] = W1e[j*128 + p, i*128 + c]
    return np.ascontiguousarray(
        W1e.reshape(D_T, P, F_T, P).transpose(1, 2, 0, 3).astype(W_NP)
    )


def _pack_w2(W2e):
    # w2img[p, k, i, c] = W2e[i*128 + p, k*128 + c]
    return np.ascontiguousarray(
        W2e.reshape(F_T, P, O_T, P).transpose(1, 2, 0, 3).astype(W_NP)
    )


def _shard(x, groups, W1, b1, W2, b2):
    idx = np.asarray(groups)[:, GROUP_COL].astype(np.int64)
    order = np.argsort(idx, kind="stable")
    counts = np.bincount(idx, minlength=E)
    cap, n_chunks, chunk = _plan_capacity(counts.max())
    offs = np.concatenate([[0], np.cumsum(counts)])

    x = np.asarray(x, dtype=np.float32)
    W1 = np.asarray(W1, dtype=np.float32)
    b1 = np.asarray(b1, dtype=np.float32)
    W2 = np.asarray(W2, dtype=np.float32)
    b2 = np.asarray(b2, dtype=np.float32)

    in_maps, tok_ids = [], []
    for e in range(E):
        ids = order[offs[e] : offs[e + 1]]
        tok_ids.append(ids)
        xT = np.zeros((D_IN, cap), np.float32)
        xT[:, : len(ids)] = x[ids].T
        # pack each chunk to its SBUF image [p, j, c] so every chunk DMA
        # moves multi-KB contiguous lines on both sides
        xT = np.ascontiguousarray(
            xT.reshape(D_T, P, n_chunks, chunk)
            .transpose(2, 1, 0, 3)
            .astype(W_NP)
        )
        bt = np.concatenate(
            [b1[e].reshape(F_T, P).T, b2[e].reshape(O_T, P).T], axis=1
        )
        in_maps.append(
            {
                "xT": xT,
                "w1": _pack_w1(W1[e]),
                "w2": _pack_w2(W2[e]),
                "bt": np.ascontiguousarray(bt),
            }
        )
    return in_maps, tok_ids, counts, cap, n_chunks, chunk


def _run(x, groups, W1, b1, W2, b2, trace=False, **spmd_kwargs):
    in_maps, tok_ids, counts, cap, n_chunks, chunk = _shard(x, groups, W1, b1, W2, b2)
    nc = _build_nc(cap, n_chunks, chunk)
    res = run_bass_kernel_spmd(
        nc, in_maps, core_ids=list(range(E)), trace=trace, **spmd_kwargs
    )
    out = np.zeros((B, D_OUT), np.float32)
    for e in range(E):
        # yT[c, p, k, cc] = y[k*128 + p, c*chunk + cc]
        yTe = np.asarray(res.results[e]["yT"], dtype=np.float32)
        ye = yTe.transpose(2, 1, 0, 3).reshape(D_OUT, cap)
        out[tok_ids[e]] = ye[:, : counts[e]].T
    return out, res


def kernel(x, groups, W1, b1, W2, b2):
    out, _ = _run(x, groups, W1, b1, W2, b2)
    return out


# revision 32
# speedup vs baseline: 1.1419x; 1.0187x over previous
"""nn_GateMulti — MoE routing (8 experts, one-hot gate) on 8 TRN2 NeuronCores.

Strategy: expert-parallel. The gate is exactly one-hot on groups[:, 0], so
each token needs exactly one expert's MLP. Host-side "all-to-all": sort the
4096 tokens by expert id, pad each expert's token set to a common capacity,
and hand core e exactly expert e's tokens (transposed) plus expert e's
weights. Each core then runs a dense 2-layer MLP:

    yT = W2.T @ relu(W1.T @ xT + b1) + b2        (feature-major layout)

All matmul operands are bf16 (half the DMA bytes, fast-weight-load); PSUM
accumulation is fp32. The PE stream is the roofline: 2*128 matmuls of
N=chunk columns at ~N/2.4GHz+3ns each. Everything else is arranged around
keeping the PE fed:

- Input DMAs go on the sync HWDGE ring in exact consumption order and are
  chained (transfer k waits for transfer k-4): queued DMAs round-robin at
  packet granularity across the 16 SDMA engines, so an unchained deep
  queue stretches every transfer's completion semaphore by multiple us.
  Chaining on the DMAs themselves keeps delivery at wire speed regardless
  of PE pace. Biases ride the scalar ring.
- A burst of dependency-free warm-up matmuls covers the preamble->pilot
  window so the PE HAM clock gate reaches 8/8 when real work starts, and
  cushions wire-ramp variance (a late pilot otherwise idles the PE long
  enough for the HAM MID window to re-throttle the early stream).
- Layer 1 runs chunk-interleaved with the first two f-tiles chunk-0-first,
  so xT chunk 1 (3rd transfer on the wire) is not needed until ~2
  tile-times into the stream. ReLU+bias tiles alternate between ScalarE
  (activation) and DVE (tensor_scalar add+max) so neither engine gates
  the PE's PSUM banks.
- y leaves in bf16 (host upcasts) packed [P, O_T, chunk] per chunk so
  out-DMA lines are 2 KB; the final o-tile goes out as its own small DMA
  on the scalar ring right after a ScalarE-only final ACT, keeping the
  tail to ACT + one issue + receipt.

The host scatters per-core outputs back to the original token order.
Self-contained: shapes hardcoded from the problem spec.
"""

import math
from functools import lru_cache

import ml_dtypes
import numpy as np

import concourse.bacc as bacc
import concourse.mybir as mybir
import concourse.tile as tile
from concourse.bass_utils import run_bass_kernel_spmd

E = 8
B = 4096
D_IN = 512
D_FF = 2048
D_OUT = 512
GROUP_COL = 0

P = 128
D_T = D_IN // P   # 4  k-tiles for layer 1
F_T = D_FF // P   # 16 f-tiles (layer-1 out / layer-2 contraction)
O_T = D_OUT // P  # 4  o-tiles for layer 2

F32 = mybir.dt.float32
BF16 = mybir.dt.bfloat16
W_DT = A_DT = BF16
W_NP = ml_dtypes.bfloat16

N_WARM = 16  # dependency-free scratch matmuls spanning preamble-end ->
             # pilot-sem-firing (~10.5-11us at ~226ns cold each); also
             # lifts the HAM clock gate to 8/8 by the time the real stream
             # starts, and cushions run-to-run wire-ramp variance (a late
             # pilot otherwise leaves the PE idle long enough for the HAM
             # MID window to re-throttle the early stream to 1.2 GHz)

# w1 f-tile DMA slicing. Input DMAs are chained: transfer k waits for
# transfer k-CHAIN_DEPTH to complete, so at most ~3 share the wire at
# once. Queued DMAs round-robin at packet granularity across the 16 SDMA
# engines, so an unchained deep queue stretches every transfer's
# completion by multiple us; chaining on the DMAs themselves (rather than
# compute progress) keeps delivery at wire speed regardless of PE pace.
W1_SLICES = [(0, 2), (2, 4), (4, 6), (6, 8), (8, 11), (11, 16)]
CHAIN_DEPTH = 4


def _emit(tc, nc, xT, w1, w2, bt, yT, cap, n_chunks, chunk):
    relu = mybir.ActivationFunctionType.Relu
    ident = mybir.ActivationFunctionType.Identity
    add = mybir.AluOpType.add
    amax = mybir.AluOpType.max
    from concourse.bass import _add_dep_helper

    with (
        tc.tile_pool(name="consts", bufs=1) as cpool,
        tc.tile_pool(name="acts", bufs=1) as apool,
        tc.tile_pool(name="yout", bufs=1) as ypool,
        tc.tile_pool(name="psum_h", bufs=4, space="PSUM") as ph,
        tc.tile_pool(name="psum_y", bufs=4, space="PSUM") as py,
    ):
        # ---- PE warm-up: scratch matmuls with no input dependencies. They
        # hold the HAM activity window busy during the pilot-DMA wait so the
        # real stream reaches 2.4 GHz quickly, sized to not overshoot it.
        warm_w = cpool.tile([P, P], W_DT)
        warm_x = cpool.tile([P, chunk], A_DT)
        nc.vector.memset(warm_w[:], 0.0)
        nc.vector.memset(warm_x[:], 0.0)
        warm_p = py.tile([P, chunk], F32, name="warm_p", tag="yp")
        for _ in range(N_WARM):
            nc.tensor.matmul(warm_p[:], warm_w[:], warm_x[:])

        # ---- input DMAs on the sync ring in consumption order (xT chunk 0
        # + first w1 pair lead as the pilot), chained at CHAIN_DEPTH so the
        # wire is never deep enough for completion-semaphore straggle;
        # biases ride the scalar ring concurrently.
        w1_sb = cpool.tile([P, F_T, D_T, P], W_DT)   # [p, i, j, c]
        w2_sb = cpool.tile([P, O_T, F_T, P], W_DT)   # [p, k, i, c]
        xT_sb = apool.tile([P, n_chunks, D_T, chunk], A_DT)
        bt_sb = cpool.tile([P, F_T + O_T], F32)

        nc.scalar.dma_start(bt_sb[:], bt.ap())
        chain = []
        chain.append(nc.sync.dma_start(xT_sb[:, 0], xT.ap()[0]))
        chain.append(nc.sync.dma_start(w1_sb[:, 0:2], w1.ap()[:, 0:2]))
        for c in range(1, n_chunks):
            chain.append(nc.sync.dma_start(xT_sb[:, c], xT.ap()[c]))
        for lo, hi in W1_SLICES[1:]:
            chain.append(nc.sync.dma_start(w1_sb[:, lo:hi], w1.ap()[:, lo:hi]))
        chain.append(nc.sync.dma_start(w2_sb[:, 0:2], w2.ap()[:, 0:2]))
        chain.append(nc.sync.dma_start(w2_sb[:, 2:4], w2.ap()[:, 2:4]))
        for k in range(CHAIN_DEPTH, len(chain)):
            _add_dep_helper(
                chain[k].ins,
                chain[k - CHAIN_DEPTH].ins,
                sync=True,
                reason="cap in-flight input DMAs at CHAIN_DEPTH",
            )

        hT_sb = apool.tile([P, F_T, cap], A_DT)

        # ---- layer 1: hT[f, c] = relu(sum_d W1[d, f] xT[d, c] + b1[f])
        # chunk-interleaved (i outer) so each w1 f-tile is consumed at half
        # rate, keeping every w1 DMA's completion comfortably ahead of its
        # consumer; the first two i-tiles run chunk-0-first so xT chunk 1
        # is not needed until ~2 tile-times into the stream (it is the 3rd
        # transfer on the wire). ReLU tiles alternate ScalarE / DVE so
        # neither engine backs up PSUM.
        l1_order = [(i, c) for i in range(F_T) for c in range(n_chunks)]
        if F_T >= 2 and n_chunks == 2:
            l1_order[0:4] = [(0, 0), (1, 0), (0, 1), (1, 1)]
        for i, c in l1_order:
            cs = slice(c * chunk, (c + 1) * chunk)
            hp = ph.tile([P, chunk], F32, name=f"hp_{i}_{c}", tag="hp")
            for j in range(D_T):
                nc.tensor.matmul(
                    hp[:],
                    w1_sb[:, i, j, :],
                    xT_sb[:, c, j, :],
                    start=(j == 0),
                    stop=(j == D_T - 1),
                )
            if (i + c) % 2 == 0:
                nc.scalar.activation(
                    hT_sb[:, i, cs], hp[:], relu, bias=bt_sb[:, i : i + 1]
                )
            else:
                nc.vector.tensor_scalar(
                    hT_sb[:, i, cs], hp[:], bt_sb[:, i : i + 1], 0.0, add, amax
                )

        # ---- layer 2: yT[o, c] = sum_f W2[f, o] hT[f, c] + b2[o]
        # c outer so each chunk's 4 o-tiles pack into one [P, O_T, chunk]
        # SBUF tile -> one 2KB-line DMA per chunk. The final o-tile of the
        # last chunk is ACT-split across ScalarE+DVE and leaves as its own
        # small DMA on the scalar ring so the tail is as short as possible.
        for c in range(n_chunks):
            cs = slice(c * chunk, (c + 1) * chunk)
            yo = ypool.tile([P, O_T, chunk], A_DT, name=f"yo_{c}", tag=f"yo{c}")
            last_c = c == n_chunks - 1
            for k in range(O_T):
                yp = py.tile([P, chunk], F32, name=f"yp_{k}_{c}", tag="yp")
                for i in range(F_T):
                    nc.tensor.matmul(
                        yp[:],
                        w2_sb[:, k, i, :],
                        hT_sb[:, i, cs],
                        start=(i == 0),
                        stop=(i == F_T - 1),
                    )
                b2c = bt_sb[:, F_T + k : F_T + k + 1]
                last = last_c and k == O_T - 1
                if not last:
                    nc.vector.tensor_scalar_add(yo[:, k], yp[:], b2c)
                else:
                    # final tile on ScalarE alone: splitting it across
                    # ScalarE+DVE does not help — consumers of one PSUM
                    # bank serialize anyway — and its DMA follows on the
                    # same queue with no cross-engine semaphore hop
                    nc.scalar.activation(yo[:, k], yp[:], ident, bias=b2c)
            if not last_c:
                nc.sync.dma_start(yT[c], yo[:])
            else:
                # split the last chunk's out-DMA: o-tiles 0..2 on sync once
                # ready, the final o-tile alone on the scalar ring
                nc.sync.dma_start(yT[c, :, 0 : O_T - 1], yo[:, 0 : O_T - 1])
                nc.scalar.dma_start(yT[c, :, O_T - 1 :], yo[:, O_T - 1 :])


@lru_cache(maxsize=4)
def _build_nc(cap, n_chunks, chunk):
    nc = bacc.Bacc("TRN2", target_bir_lowering=False, debug=False, num_devices=E)
    xT = nc.dram_tensor("xT", [n_chunks, P, D_T, chunk], A_DT, kind="ExternalInput")
    w1 = nc.dram_tensor("w1", [P, F_T, D_T, P], W_DT, kind="ExternalInput")
    w2 = nc.dram_tensor("w2", [P, O_T, F_T, P], W_DT, kind="ExternalInput")
    bt = nc.dram_tensor("bt", [P, F_T + O_T], F32, kind="ExternalInput")
    yT = nc.dram_tensor("yT", [n_chunks, P, O_T, chunk], A_DT, kind="ExternalOutput")
    with tile.TileContext(nc) as tc:
        _emit(tc, nc, xT, w1, w2, bt, yT, cap, n_chunks, chunk)
    nc.compile()
    return nc


def _plan_capacity(max_count):
    cap0 = max(int(max_count), 16)
    n_chunks = max(1, math.ceil(cap0 / 512))
    chunk = math.ceil(cap0 / (n_chunks * 2)) * 2
    return n_chunks * chunk, n_chunks, chunk


def _pack_w1(W1e):
    # w1img[p, i, j, c] = W1e[j*128 + p, i*128 + c]
    return np.ascontiguousarray(
        W1e.reshape(D_T, P, F_T, P).transpose(1, 2, 0, 3).astype(W_NP)
    )


def _pack_w2(W2e):
    # w2img[p, k, i, c] = W2e[i*128 + p, k*128 + c]
    return np.ascontiguousarray(
        W2e.reshape(F_T, P, O_T, P).transpose(1, 2, 0, 3).astype(W_NP)
    )


def _shard(x, groups, W1, b1, W2, b2):
    idx = np.asarray(groups)[:, GROUP_COL].astype(np.int64)
    order = np.argsort(idx, kind="stable")
    counts = np.bincount(idx, minlength=E)
    cap, n_chunks, chunk = _plan_capacity(counts.max())
    offs = np.concatenate([[0], np.cumsum(counts)])

    x = np.asarray(x, dtype=np.float32)
    W1 = np.asarray(W1, dtype=np.float32)
    b1 = np.asarray(b1, dtype=np.float32)
    W2 = np.asarray(W2, dtype=np.float32)
    b2 = np.asarray(b2, dtype=np.float32)

    in_maps, tok_ids = [], []
    for e in range(E):
        ids = order[offs[e] : offs[e + 1]]
        tok_ids.append(ids)
        xT = np.zeros((D_IN, cap), np.float32)
        xT[:, : len(ids)] = x[ids].T
        # pack each chunk to its SBUF image [p, j, c] so every chunk DMA
        # moves multi-KB contiguous lines on both sides
        xT = np.ascontiguousarray(
            xT.reshape(D_T, P, n_chunks, chunk)
            .transpose(2, 1, 0, 3)
            .astype(W_NP)
        )
        bt = np.concatenate(
            [b1[e].reshape(F_T, P).T, b2[e].reshape(O_T, P).T], axis=1
        )
        in_maps.append(
            {
                "xT": xT,
                "w1": _pack_w1(W1[e]),
                "w2": _pack_w2(W2[e]),
                "bt": np.ascontiguousarray(bt),
            }
        )
    return in_maps, tok_ids, counts, cap, n_chunks, chunk


def _run(x, groups, W1, b1, W2, b2, trace=False, **spmd_kwargs):
    in_maps, tok_ids, counts, cap, n_chunks, chunk = _shard(x, groups, W1, b1, W2, b2)
    nc = _build_nc(cap, n_chunks, chunk)
    res = run_bass_kernel_spmd(
        nc, in_maps, core_ids=list(range(E)), trace=trace, **spmd_kwargs
    )
    out = np.zeros((B, D_OUT), np.float32)
    for e in range(E):
        # yT[c, p, k, cc] = y[k*128 + p, c*chunk + cc]
        yTe = np.asarray(res.results[e]["yT"], dtype=np.float32)
        ye = yTe.transpose(2, 1, 0, 3).reshape(D_OUT, cap)
        out[tok_ids[e]] = ye[:, : counts[e]].T
    return out, res


def kernel(x, groups, W1, b1, W2, b2):
    out, _ = _run(x, groups, W1, b1, W2, b2)
    return out
